# revision 1
# baseline (speedup 1.0000x reference)
"""GQA kernel for trn2: 8 NeuronCores, SPMD (b in {0,1} x 4 head-groups).

Per core (b, hg): 8 q-heads (8hg..8hg+7), 2 kv-heads (2hg, 2hg+1).
f32r matmuls for projections/scores/out-proj, bf16 for attn@V.
Host preps transposed/permuted weights; partial outputs summed on host
(row-parallel Wo all-reduce done during unshard).
"""
import numpy as np
import ml_dtypes
import concourse.bass as bass
import concourse.mybir as mybir
from concourse import tile, bacc
from concourse.bass_utils import run_bass_kernel_spmd

B, S, D = 2, 2048, 2048
H, KVH, DH = 32, 8, 64
SCALE = DH ** -0.5
SC = 4          # Sq chunks of 512
KD = 16         # D contraction chunks of 128
NJ = 16         # Sk blocks of 128
F32R = mybir.dt.float32r
F32 = mybir.dt.float32
BF16 = mybir.dt.bfloat16

_cache = {}


def build():
    nc = bacc.Bacc('TRN2', target_bir_lowering=False, debug=False, num_devices=8)
    xT_p = nc.declare_dram_parameter('xT', [D, S], F32R, isOutput=False)
    WT_p = nc.declare_dram_parameter('WT', [D, 768], F32R, isOutput=False)
    WoT_p = nc.declare_dram_parameter('WoT', [512, D], F32R, isOutput=False)
    cos4_p = nc.declare_dram_parameter('cos4', [128, S], F32, isOutput=False)
    sin4_p = nc.declare_dram_parameter('sin4', [128, S], F32, isOutput=False)
    mask_p = nc.declare_dram_parameter('mask', [128, 4 * 512], BF16, isOutput=False)
    ident_p = nc.declare_dram_parameter('ident', [128, 128], F32, isOutput=False)
    out_p = nc.declare_dram_parameter('out', [S, D], F32, isOutput=True)

    with tile.TileContext(nc) as tc:
        with tc.tile_pool(name='w', bufs=1) as wpool, \
             tc.tile_pool(name='x', bufs=17) as xpool, \
             tc.tile_pool(name='qk', bufs=1) as qkpool, \
             tc.tile_pool(name='tmp', bufs=2) as tpool, \
             tc.tile_pool(name='at', bufs=4) as atpool, \
             tc.tile_pool(name='acc', bufs=1, space='PSUM') as accp, \
             tc.tile_pool(name='scp', bufs=4, space='PSUM') as scp, \
             tc.tile_pool(name='pop', bufs=1, space='PSUM') as pop:

            WT = wpool.tile([128, KD * 768], F32R, tag='WTshare', name='WT')
            for kd in range(KD):
                nc.sync.dma_start(out=WT[:, kd * 768:(kd + 1) * 768],
                                  in_=WT_p[128 * kd:128 * (kd + 1), :])
            cos4 = wpool.tile([128, S], F32, tag='cs', name='cos4', bufs=2)
            sin4 = wpool.tile([128, S], F32)
            masks = wpool.tile([128, 4 * 512], BF16)
            ident = wpool.tile([128, 128], F32)
            nc.sync.dma_start(out=cos4[:], in_=cos4_p[:])
            nc.sync.dma_start(out=sin4[:], in_=sin4_p[:])
            nc.sync.dma_start(out=masks[:], in_=mask_p[:])
            nc.sync.dma_start(out=ident[:], in_=ident_p[:])

            # ---- projections + RoPE ----
            qk = [qkpool.tile([128, S], F32R, tag=f'qk{m}', name=f'qk{m}') for m in range(5)]
            vT = qkpool.tile([128, S], F32, tag='vT')
            for s in range(SC):
                xts = []
                for kd in range(KD):
                    xt = xpool.tile([128, 512], F32R, tag='xt')
                    nc.sync.dma_start(out=xt[:],
                                      in_=xT_p[128 * kd:128 * (kd + 1), 512 * s:512 * (s + 1)])
                    xts.append(xt)
                for mg in range(3):
                    psmg = accp.tile([128, 1024], F32, tag='acc')
                    for m in (2 * mg, 2 * mg + 1):
                        ps = psmg[:, 512 * (m - 2 * mg):512 * (m - 2 * mg) + 512]
                        for kd in range(KD):
                            nc.tensor.matmul(ps[:], WT[:, kd * 768 + 128 * m:kd * 768 + 128 * (m + 1)],
                                             xts[kd][:], start=(kd == 0), stop=(kd == KD - 1))
                        if m < 5:
                            # RoPE: out = ps*cos4 + swap32(ps)*sin4 (sign baked in sin4)
                            t1 = tpool.tile([128, 512], F32, tag='t1')
                            t2 = tpool.tile([128, 512], F32, tag='t2')
                            nc.vector.tensor_mul(t1[:], ps[:], cos4[:, 512 * s:512 * (s + 1)])
                            for g in range(2):
                                b0 = 64 * g
                                nc.vector.tensor_mul(t2[b0:b0 + 32, :], ps[b0 + 32:b0 + 64, :],
                                                     sin4[b0:b0 + 32, 512 * s:512 * (s + 1)])
                                nc.vector.tensor_mul(t2[b0 + 32:b0 + 64, :], ps[b0:b0 + 32, :],
                                                     sin4[b0 + 32:b0 + 64, 512 * s:512 * (s + 1)])
                            nc.vector.tensor_add(qk[m][:, 512 * s:512 * (s + 1)], t1[:], t2[:])
                        else:
                            nc.vector.tensor_copy(vT[:, 512 * s:512 * (s + 1)], ps[:])

            # ---- kT replication: rep0=[kv0|kv0], rep1=[kv1|kv1] ----
            kswap = qkpool.tile([128, S], F32R, tag='kswap')
            nc.sync.dma_start(out=kswap[0:64, :], in_=qk[4][64:128, :])
            nc.sync.dma_start(out=kswap[64:128, :], in_=qk[4][0:64, :])

            # ---- V natural (bf16, ones-augmented): per j [v0 64|1|v1 64|1] ----
            V = qkpool.tile([128, NJ * 130], BF16, tag='V')
            for j in range(NJ):
                pt = scp.tile([128, 128], F32, tag='sc')
                nc.tensor.transpose(pt[:], vT[:, 128 * j:128 * (j + 1)], ident[:])
                nc.vector.tensor_copy(V[:, 130 * j:130 * j + 64], pt[:, 0:64])
                nc.vector.tensor_copy(V[:, 130 * j + 65:130 * j + 129], pt[:, 64:128])
                nc.vector.memset(V[:, 130 * j + 64:130 * j + 65], 1.0)
                nc.vector.memset(V[:, 130 * j + 129:130 * j + 130], 1.0)

            # ---- WoT load (overlaps attention) ----
            WoT = wpool.tile([128, 4 * D], F32R, tag='WTshare')
            for hc in range(4):
                nc.sync.dma_start(out=WoT[:, hc * D:(hc + 1) * D],
                                  in_=WoT_p[128 * hc:128 * (hc + 1), :])

            # ---- attention ----
            aout = [qkpool.tile([128, S], F32R, tag=('ao0' if i == 0 else f'qk{i-1}'), name=f'ao{i}') for i in range(4)]
            for hp in range(4):
                kv = hp // 2
                vbase = 65 * kv
                for c in range(SC):
                    oA = pop.tile([65, 512], F32, tag='oA')
                    oB = pop.tile([65, 512], F32, tag='oB')
                    nj = 4 * c + 4
                    for j in range(nj):
                        psa = scp.tile([128, 512], F32, tag='sc', name='psa')
                        psb = scp.tile([128, 512], F32, tag='sc', name='psb')
                        kA = qk[4][0:64, 128 * j:128 * (j + 1)] if kv == 0 else kswap[0:64, 128 * j:128 * (j + 1)]
                        kB = kswap[64:128, 128 * j:128 * (j + 1)] if kv == 0 else qk[4][64:128, 128 * j:128 * (j + 1)]
                        nc.tensor.matmul(psa[:], kA,
                                         qk[hp][0:64, 512 * c:512 * (c + 1)], start=True, stop=True)
                        nc.tensor.matmul(psb[:], kB,
                                         qk[hp][64:128, 512 * c:512 * (c + 1)], start=True, stop=True)
                        ata = atpool.tile([128, 512], BF16, tag='at', name='ata')
                        atb = atpool.tile([128, 512], BF16, tag='at', name='atb')
                        nc.scalar.activation(ata[:], psa[:], mybir.ActivationFunctionType.Exp,
                                             scale=SCALE)
                        nc.scalar.activation(atb[:], psb[:], mybir.ActivationFunctionType.Exp,
                                             scale=SCALE)
                        d = j - 4 * c
                        if 0 <= d <= 3:
                            nc.vector.tensor_mul(ata[:], ata[:], masks[:, 512 * d:512 * (d + 1)])
                            nc.vector.tensor_mul(atb[:], atb[:], masks[:, 512 * d:512 * (d + 1)])
                        nc.tensor.matmul(oA[:], V[:, 130 * j + vbase:130 * j + vbase + 65],
                                         ata[:], start=(j == 0), stop=(j == nj - 1))
                        nc.tensor.matmul(oB[:], V[:, 130 * j + vbase:130 * j + vbase + 65],
                                         atb[:], start=(j == 0), stop=(j == nj - 1))
                    rA = tpool.tile([1, 512], F32, tag='rA')
                    rB = tpool.tile([1, 512], F32, tag='rB')
                    nc.vector.reciprocal(rA[:], oA[64:65, :])
                    nc.vector.reciprocal(rB[:], oB[64:65, :])
                    bA = tpool.tile([128, 512], F32, tag='bA')
                    bB = tpool.tile([128, 512], F32, tag='bB')
                    nc.gpsimd.partition_broadcast(bA[:], rA[0:1, :])
                    nc.gpsimd.partition_broadcast(bB[:], rB[0:1, :])
                    nc.vector.tensor_mul(aout[hp][0:64, 512 * c:512 * (c + 1)],
                                         oA[0:64, :], bA[0:64, :])
                    nc.vector.tensor_mul(aout[hp][64:128, 512 * c:512 * (c + 1)],
                                         oB[0:64, :], bB[64:128, :])

            # ---- output projection: out[s_blk, :] = sum_hc aoutT_hc @ WoT_hc ----
            for sb in range(NJ):
                ost = wpool.tile([128, D], F32, tag='cs', name='ost', bufs=2)
                for dg in range(2):
                    po = accp.tile([128, 1024], F32, tag='acc')
                    for hc in range(4):
                        for dc in (2 * dg, 2 * dg + 1):
                            nc.tensor.matmul(po[:, 512 * (dc - 2 * dg):512 * (dc - 2 * dg) + 512],
                                             aout[hc][:, 128 * sb:128 * (sb + 1)],
                                             WoT[:, hc * D + 512 * dc:hc * D + 512 * (dc + 1)],
                                             start=(hc == 0), stop=(hc == 3))
                    nc.vector.tensor_copy(ost[:, 1024 * dg:1024 * (dg + 1)], po[:])
                nc.sync.dma_start(out=out_p[128 * sb:128 * (sb + 1), :], in_=ost[:])
    nc.compile()
    return nc


_PERM = np.concatenate([np.arange(0, DH, 2), np.arange(1, DH, 2)])


def _prep_core(x, Wq, Wk, Wv, Wo, cos, sin, b, hg):
    xT = np.ascontiguousarray(x[b].T.astype(np.float32))
    # q heads 8hg..8hg+7 permuted, kv heads 2hg,2hg+1 (k permuted, v natural)
    wq = Wq.reshape(H, DH, D)[8 * hg:8 * hg + 8][:, _PERM, :].reshape(512, D)
    wk = Wk.reshape(KVH, DH, D)[2 * hg:2 * hg + 2][:, _PERM, :].reshape(128, D)
    wv = Wv.reshape(KVH, DH, D)[2 * hg:2 * hg + 2].reshape(128, D)
    WT = np.ascontiguousarray(np.concatenate([wq, wk, wv], 0).T.astype(np.float32))
    WoT = np.ascontiguousarray(Wo[:, 512 * hg:512 * (hg + 1)].T.astype(np.float32))
    cosT = np.ascontiguousarray(cos.T.astype(np.float32))          # (32, S)
    sinT = np.ascontiguousarray(sin.T.astype(np.float32))
    cos4 = np.tile(cosT, (4, 1))
    sin4 = np.concatenate([-sinT, sinT, -sinT, sinT], 0)
    mask = np.zeros((128, 4 * 512), dtype=np.float64)
    for dd in range(4):
        mask[:, 512 * dd:512 * (dd + 1)] = \
            (128 * dd + np.arange(128)[:, None]) <= np.arange(512)[None, :]
    return {'xT': xT, 'WT': WT, 'WoT': WoT, 'cos4': cos4, 'sin4': sin4,
            'mask': mask.astype(ml_dtypes.bfloat16),
            'ident': np.eye(128, dtype=np.float32)}


def _run(inputs, trace=False, tmpdir=None):
    if 'nc' not in _cache:
        _cache['nc'] = build()
    in_maps = [_prep_core(inputs['x'], inputs['Wq'], inputs['Wk'], inputs['Wv'],
                          inputs['Wo'], inputs['cos'], inputs['sin'], c // 4, c % 4)
               for c in range(8)]
    res = run_bass_kernel_spmd(_cache['nc'], in_maps, core_ids=list(range(8)),
                               trace=trace, tmpdir=tmpdir)
    parts = [res.results[c]['out'] for c in range(8)]
    out = np.stack([parts[0] + parts[1] + parts[2] + parts[3],
                    parts[4] + parts[5] + parts[6] + parts[7]], 0)
    return out.astype(np.float32), res


def kernel(**inputs):
    out, _ = _run(inputs, trace=False)
    return out



# revision 2
# speedup vs baseline: 1.3786x; 1.3786x over previous
"""GQA kernel for trn2: 8 NeuronCores, SPMD (b in {0,1} x 4 head-groups).

Per core (b, hg): 8 q-heads (8hg..8hg+7), 2 kv-heads (2hg, 2hg+1).
c-major software pipeline: per 512-wide q chunk c emit
  proj(c) -> RoPE -> V-build -> attention(c) with outproj(c-1) interleaved
so PE never sees a phase barrier (keeps HAM un-throttled).
f32r matmuls for proj/scores, bf16 for attn@V, fp16 for outproj.
Host preps transposed/permuted weights; partial outputs summed on host
(row-parallel Wo all-reduce done during unshard).
"""
import numpy as np
import ml_dtypes
import concourse.bass as bass
import concourse.mybir as mybir
from concourse import tile, bacc
from concourse.bass_utils import run_bass_kernel_spmd

B, S, D = 2, 2048, 2048
H, KVH, DH = 32, 8, 64
SCALE = DH ** -0.5
SC = 4          # Sq chunks of 512
KD = 16         # D contraction chunks of 128
NJ = 16         # Sk blocks of 128
F32R = mybir.dt.float32r
F32 = mybir.dt.float32
BF16 = mybir.dt.bfloat16
F16 = mybir.dt.float16

_cache = {}


def build():
    nc = bacc.Bacc('TRN2', target_bir_lowering=False, debug=False, num_devices=8)
    xT_p = nc.declare_dram_parameter('xT', [D, S], F32R, isOutput=False)
    WT_p = nc.declare_dram_parameter('WT', [D, 768], F32R, isOutput=False)
    WoT_p = nc.declare_dram_parameter('WoT', [512, D], F16, isOutput=False)
    cos4_p = nc.declare_dram_parameter('cos4', [128, S], F16, isOutput=False)
    sin4_p = nc.declare_dram_parameter('sin4', [128, S], F16, isOutput=False)
    mask_p = nc.declare_dram_parameter('mask', [128, 4 * 512], BF16, isOutput=False)
    ident_p = nc.declare_dram_parameter('ident', [128, 128], F32, isOutput=False)
    out_p = nc.declare_dram_parameter('out', [S, D], F32, isOutput=True)

    with tile.TileContext(nc) as tc:
        with tc.tile_pool(name='w', bufs=1) as wpool, \
             tc.tile_pool(name='x', bufs=17) as xpool, \
             tc.tile_pool(name='q', bufs=8) as qpool, \
             tc.tile_pool(name='ao', bufs=8) as aopool, \
             tc.tile_pool(name='v', bufs=2) as vpool, \
             tc.tile_pool(name='t', bufs=2) as tpool, \
             tc.tile_pool(name='at', bufs=3) as atpool, \
             tc.tile_pool(name='s', bufs=1) as spool, \
             tc.tile_pool(name='o', bufs=4) as opool, \
             tc.tile_pool(name='acc', bufs=2, space='PSUM') as accp, \
             tc.tile_pool(name='sc', bufs=2, space='PSUM') as scp, \
             tc.tile_pool(name='po', bufs=1, space='PSUM') as pop:

            WT = wpool.tile([128, KD * 768], F32R, tag='WT')
            for kd in range(KD):
                nc.sync.dma_start(out=WT[:, kd * 768:(kd + 1) * 768],
                                  in_=WT_p[128 * kd:128 * (kd + 1), :])
            cos4 = wpool.tile([128, S], F16, tag='cos4')
            sin4 = wpool.tile([128, S], F16, tag='sin4')
            masks = wpool.tile([128, 4 * 512], BF16, tag='masks')
            ident = wpool.tile([128, 128], F32, tag='ident')
            nc.sync.dma_start(out=cos4[:], in_=cos4_p[:])
            nc.sync.dma_start(out=sin4[:], in_=sin4_p[:])
            nc.sync.dma_start(out=masks[:], in_=mask_p[:])
            nc.sync.dma_start(out=ident[:], in_=ident_p[:])

            k4 = wpool.tile([128, S], F32R, tag='k4')
            kswap = wpool.tile([128, S], F32R, tag='kswap')
            V = wpool.tile([128, NJ * 130], BF16, tag='V')
            nc.vector.memset(V[:], 1.0)
            WoT = wpool.tile([128, 4 * D], F16, tag='WoT')

            aout_c = {}   # (c, hp) -> per-chunk fp16 attention-output tile

            def outproj_sb(cc, sb):
                for dc in range(4):
                    po = accp.tile([128, 512], F32, tag='acc', name=f'po{sb}_{dc}')
                    for hc in range(4):
                        nc.tensor.matmul(po[:], aout_c[(cc, hc)][:, 128 * (sb - 4 * cc):128 * (sb - 4 * cc) + 128],
                                         WoT[:, hc * D + 512 * dc: hc * D + 512 * (dc + 1)],
                                         start=(hc == 0), stop=(hc == 3))
                    ost = opool.tile([128, 512], F32, tag='ost')
                    if dc % 2 == 0:
                        nc.scalar.copy(ost[:], po[:])
                    else:
                        nc.vector.tensor_copy(ost[:], po[:])
                    nc.sync.dma_start(out=out_p[128 * sb:128 * (sb + 1), 512 * dc:512 * (dc + 1)],
                                      in_=ost[:])

            for c in range(SC):
                cs = slice(512 * c, 512 * (c + 1))
                # ---- projections + RoPE for chunk c ----
                xts = []
                for kd in range(KD):
                    xt = xpool.tile([128, 512], F32R, tag='xt')
                    nc.sync.dma_start(out=xt[:], in_=xT_p[128 * kd:128 * (kd + 1), cs])
                    xts.append(xt)
                qc = [qpool.tile([128, 512], F32R, tag='qc', name=f'qc{c}_{m}')
                      for m in range(4)]
                vT = vpool.tile([128, 512], F32, tag='vT')
                for m in range(6):
                    ps = accp.tile([128, 512], F32, tag='acc', name=f'ps{c}_{m}')
                    for kd in range(KD):
                        nc.tensor.matmul(ps[:], WT[:, kd * 768 + 128 * m: kd * 768 + 128 * (m + 1)],
                                         xts[kd][:], start=(kd == 0), stop=(kd == KD - 1))
                    if m < 5:
                        # RoPE: out = ps*cos4 + swap32(ps)*sin4 (sign baked in sin4)
                        t1 = tpool.tile([128, 512], F32, tag='t1')
                        t2 = tpool.tile([128, 512], F32, tag='t2')
                        nc.vector.tensor_mul(t1[:], ps[:], cos4[:, cs])
                        for g in range(2):
                            b0 = 64 * g
                            nc.vector.tensor_mul(t2[b0:b0 + 32, :], ps[b0 + 32:b0 + 64, :],
                                                 sin4[b0:b0 + 32, cs])
                            nc.vector.tensor_mul(t2[b0 + 32:b0 + 64, :], ps[b0:b0 + 32, :],
                                                 sin4[b0 + 32:b0 + 64, cs])
                        tgt = qc[m][:] if m < 4 else k4[:, cs]
                        nc.vector.tensor_add(tgt, t1[:], t2[:])
                    else:
                        nc.scalar.copy(vT[:], ps[:])
                # kT replication for chunk c: rep0=[kv0|kv0], rep1=[kv1|kv1]
                nc.sync.dma_start(out=kswap[0:64, cs], in_=k4[64:128, cs])
                nc.sync.dma_start(out=kswap[64:128, cs], in_=k4[0:64, cs])
                # V natural (bf16, ones-augmented): per j [v0 64|1|v1 64|1]
                for j in range(4 * c, 4 * c + 4):
                    pt = accp.tile([128, 128], F32, tag='acc', name=f'pt{j}')
                    nc.tensor.transpose(pt[:], vT[:, 128 * (j - 4 * c):128 * (j - 4 * c) + 128], ident[:])
                    nc.vector.tensor_copy(V[:, 130 * j:130 * j + 64], pt[:, 0:64])
                    nc.vector.tensor_copy(V[:, 130 * j + 65:130 * j + 129], pt[:, 64:128])
                if c == 0:
                    for hc in range(4):
                        nc.sync.dma_start(out=WoT[:, hc * D:(hc + 1) * D],
                                          in_=WoT_p[128 * hc:128 * (hc + 1), :])

                # ---- attention for chunk c (+ interleaved outproj of c-1) ----
                nj = 4 * c + 4
                for hp in range(4):
                    kv = hp // 2
                    vbase = 65 * kv
                    oA = pop.tile([65, 512], F32, tag='oA')
                    oB = pop.tile([65, 512], F32, tag='oB')
                    atas = {}

                    def scores(j):
                        sct = scp.tile([128, 1024], F32, tag='sc', name=f'sc{hp}_{c}_{j}')
                        if kv == 0:
                            kA = k4[0:64, 128 * j:128 * (j + 1)]
                            kB = kswap[64:128, 128 * j:128 * (j + 1)]
                        else:
                            kA = kswap[0:64, 128 * j:128 * (j + 1)]
                            kB = k4[64:128, 128 * j:128 * (j + 1)]
                        nc.tensor.matmul(sct[:, 0:512], kA, qc[hp][0:64, :], start=True, stop=True)
                        nc.tensor.matmul(sct[:, 512:1024], kB, qc[hp][64:128, :], start=True, stop=True)
                        ata = atpool.tile([128, 1024], BF16, tag='at', name=f'at{hp}_{c}_{j}')
                        nc.scalar.activation(ata[:], sct[:], mybir.ActivationFunctionType.Exp,
                                             scale=SCALE)
                        d = j - 4 * c
                        if 0 <= d <= 3:
                            nc.vector.tensor_mul(ata[:, 0:512], ata[:, 0:512],
                                                 masks[:, 512 * d:512 * (d + 1)])
                            nc.gpsimd.tensor_mul(ata[:, 512:1024], ata[:, 512:1024],
                                                 masks[:, 512 * d:512 * (d + 1)])
                        atas[j] = ata

                    def attnv(j):
                        ata = atas.pop(j)
                        nc.tensor.matmul(oA[:], V[:, 130 * j + vbase:130 * j + vbase + 65],
                                         ata[:, 0:512], start=(j == 0), stop=(j == nj - 1))
                        nc.tensor.matmul(oB[:], V[:, 130 * j + vbase:130 * j + vbase + 65],
                                         ata[:, 512:1024], start=(j == 0), stop=(j == nj - 1))

                    scores(0)
                    for j in range(1, nj):
                        scores(j)
                        attnv(j - 1)
                    attnv(nj - 1)

                    # normalize: evacuate PSUM fast, then recip/broadcast/mul
                    sbA = spool.tile([65, 512], F32, tag='sbA')
                    sbB = spool.tile([65, 512], F32, tag='sbB')
                    nc.scalar.copy(sbA[:], oA[:])
                    nc.scalar.copy(sbB[:], oB[:])
                    rA = spool.tile([1, 512], F32, tag='rA')
                    rB = spool.tile([1, 512], F32, tag='rB')
                    nc.vector.reciprocal(rA[:], sbA[64:65, :])
                    nc.vector.reciprocal(rB[:], sbB[64:65, :])
                    bA = spool.tile([64, 512], F32, tag='bA')
                    bB = spool.tile([64, 512], F32, tag='bB')
                    nc.gpsimd.partition_broadcast(bA[:], rA[0:1, :])
                    nc.gpsimd.partition_broadcast(bB[:], rB[0:1, :])
                    ao = aopool.tile([128, 512], F16, tag='ao', name=f'ao{c}_{hp}')
                    aout_c[(c, hp)] = ao
                    nc.vector.tensor_mul(ao[0:64, :], sbA[0:64, :], bA[:])
                    nc.vector.tensor_mul(ao[64:128, :], sbB[0:64, :], bB[:])

                    if c >= 1:
                        outproj_sb(c - 1, 4 * (c - 1) + hp)

            for sb in range(12, 16):
                outproj_sb(3, sb)
    nc.compile()
    return nc


_PERM = np.concatenate([np.arange(0, DH, 2), np.arange(1, DH, 2)])


def _prep_core(x, Wq, Wk, Wv, Wo, cos, sin, b, hg):
    xT = np.ascontiguousarray(x[b].T.astype(np.float32))
    # q heads 8hg..8hg+7 permuted, kv heads 2hg,2hg+1 (k permuted, v natural)
    wq = Wq.reshape(H, DH, D)[8 * hg:8 * hg + 8][:, _PERM, :].reshape(512, D)
    wk = Wk.reshape(KVH, DH, D)[2 * hg:2 * hg + 2][:, _PERM, :].reshape(128, D)
    wv = Wv.reshape(KVH, DH, D)[2 * hg:2 * hg + 2].reshape(128, D)
    WT = np.ascontiguousarray(np.concatenate([wq, wk, wv], 0).T.astype(np.float32))
    WoT = np.ascontiguousarray(Wo[:, 512 * hg:512 * (hg + 1)].T.astype(np.float16))
    cosT = np.ascontiguousarray(cos.T.astype(np.float32))          # (32, S)
    sinT = np.ascontiguousarray(sin.T.astype(np.float32))
    cos4 = np.tile(cosT, (4, 1)).astype(np.float16)
    sin4 = np.concatenate([-sinT, sinT, -sinT, sinT], 0).astype(np.float16)
    mask = np.zeros((128, 4 * 512), dtype=np.float64)
    for dd in range(4):
        mask[:, 512 * dd:512 * (dd + 1)] = \
            (128 * dd + np.arange(128)[:, None]) <= np.arange(512)[None, :]
    return {'xT': xT, 'WT': WT, 'WoT': WoT, 'cos4': cos4, 'sin4': sin4,
            'mask': mask.astype(ml_dtypes.bfloat16),
            'ident': np.eye(128, dtype=np.float32)}


def _run(inputs, trace=False, tmpdir=None):
    if 'nc' not in _cache:
        _cache['nc'] = build()
    in_maps = [_prep_core(inputs['x'], inputs['Wq'], inputs['Wk'], inputs['Wv'],
                          inputs['Wo'], inputs['cos'], inputs['sin'], c // 4, c % 4)
               for c in range(8)]
    res = run_bass_kernel_spmd(_cache['nc'], in_maps, core_ids=list(range(8)),
                               trace=trace, tmpdir=tmpdir)
    parts = [res.results[c]['out'] for c in range(8)]
    out = np.stack([parts[0] + parts[1] + parts[2] + parts[3],
                    parts[4] + parts[5] + parts[6] + parts[7]], 0)
    return out.astype(np.float32), res


def kernel(**inputs):
    out, _ = _run(inputs, trace=False)
    return out


# revision 8
# speedup vs baseline: 1.4326x; 1.0392x over previous
"""GQA kernel for trn2: 8 NeuronCores, SPMD (b in {0,1} x 4 head-groups).

Per core (b, hg): 8 q-heads (8hg..8hg+7), 2 kv-heads (2hg, 2hg+1).
c-major software pipeline: per 512-wide q chunk c emit
  proj(c) -> RoPE -> V-build -> attention(c) with outproj(c-1) interleaved
so PE never sees a phase barrier (keeps HAM un-throttled).
f32r matmuls for proj/scores, bf16 for attn@V, fp16 for outproj.
Host preps transposed/permuted weights; partial outputs summed on host
(row-parallel Wo all-reduce done during unshard).
"""
import numpy as np
import ml_dtypes
import concourse.bass as bass
import concourse.mybir as mybir
from concourse import tile, bacc
from concourse.bass_utils import run_bass_kernel_spmd

B, S, D = 2, 2048, 2048
H, KVH, DH = 32, 8, 64
SCALE = DH ** -0.5
SC = 4          # Sq chunks of 512
KD = 16         # D contraction chunks of 128
NJ = 16         # Sk blocks of 128
F32R = mybir.dt.float32r
F32 = mybir.dt.float32
BF16 = mybir.dt.bfloat16
F16 = mybir.dt.float16

_cache = {}


def build():
    nc = bacc.Bacc('TRN2', target_bir_lowering=False, debug=False, num_devices=8)
    xT_p = nc.declare_dram_parameter('xT', [D, S], F32R, isOutput=False)
    WT_p = nc.declare_dram_parameter('WT', [D, 768], F32R, isOutput=False)
    WoT_p = nc.declare_dram_parameter('WoT', [512, D], F16, isOutput=False)
    cos4_p = nc.declare_dram_parameter('cos4', [128, S], F16, isOutput=False)
    sin4_p = nc.declare_dram_parameter('sin4', [128, S], F16, isOutput=False)
    mask_p = nc.declare_dram_parameter('mask', [128, 4 * 512], BF16, isOutput=False)
    ident_p = nc.declare_dram_parameter('ident', [128, 128], F32, isOutput=False)
    out_p = nc.declare_dram_parameter('out', [S, D], F32, isOutput=True)

    with tile.TileContext(nc) as tc:
        with tc.tile_pool(name='w', bufs=1) as wpool, \
             tc.tile_pool(name='x', bufs=18) as xpool, \
             tc.tile_pool(name='q', bufs=8) as qpool, \
             tc.tile_pool(name='ao', bufs=8) as aopool, \
             tc.tile_pool(name='v', bufs=2) as vpool, \
             tc.tile_pool(name='t', bufs=2) as tpool, \
             tc.tile_pool(name='at', bufs=3) as atpool, \
             tc.tile_pool(name='s', bufs=1) as spool, \
             tc.tile_pool(name='o', bufs=2) as opool, \
             tc.tile_pool(name='acc', bufs=2, space='PSUM') as accp, \
             tc.tile_pool(name='sc', bufs=2, space='PSUM') as scp, \
             tc.tile_pool(name='po', bufs=1, space='PSUM') as pop:

            WT = wpool.tile([128, KD * 768], F32R, tag='WT')
            for kd in range(KD):
                nc.sync.dma_start(out=WT[:, kd * 768:(kd + 1) * 768],
                                  in_=WT_p[128 * kd:128 * (kd + 1), :])
            cos4 = wpool.tile([128, S], F16, tag='cos4')
            sin4 = wpool.tile([128, S], F16, tag='sin4')
            masks = wpool.tile([128, 4 * 512], BF16, tag='masks')
            ident = wpool.tile([128, 128], F32, tag='ident')
            nc.sync.dma_start(out=cos4[:], in_=cos4_p[:])
            nc.sync.dma_start(out=sin4[:], in_=sin4_p[:])
            nc.sync.dma_start(out=masks[:], in_=mask_p[:])
            nc.sync.dma_start(out=ident[:], in_=ident_p[:])

            k4 = wpool.tile([128, S], F32R, tag='k4')
            kswap = wpool.tile([128, S], F32R, tag='kswap')
            V = wpool.tile([128, NJ * 130], BF16, tag='V')
            nc.vector.memset(V[:], 1.0)
            WoT = wpool.tile([128, 4 * D], F16, tag='WoT')

            aout_c = {}   # (c, hp) -> per-chunk fp16 attention-output tile

            def outproj_unit(cc, sb, dg):
                # one (s-block, 1024-wide D group): 8 matmuls + 2 evacs + 1 DMA
                ost = opool.tile([128, 1024], F32, tag='ost')
                for i, dc in enumerate((2 * dg, 2 * dg + 1)):
                    po = accp.tile([128, 512], F32, tag='acc', name=f'po{sb}_{dc}')
                    for hc in range(4):
                        nc.tensor.matmul(po[:], aout_c[(cc, hc)][:, 128 * (sb - 4 * cc):128 * (sb - 4 * cc) + 128],
                                         WoT[:, hc * D + 512 * dc: hc * D + 512 * (dc + 1)],
                                         start=(hc == 0), stop=(hc == 3))
                    if dc % 2 == 0:
                        nc.scalar.copy(ost[:, 512 * i:512 * (i + 1)], po[:])
                    else:
                        nc.vector.tensor_copy(ost[:, 512 * i:512 * (i + 1)], po[:])
                nc.sync.dma_start(out=out_p[128 * sb:128 * (sb + 1), 1024 * dg:1024 * (dg + 1)],
                                  in_=ost[:])

            pending = []  # deferred outproj units of the previous chunk

            for c in range(SC):
                cs = slice(512 * c, 512 * (c + 1))
                # ---- projections + RoPE for chunk c ----
                xts = []
                for kd in range(KD):
                    xt = xpool.tile([128, 512], F32R, tag='xt')
                    nc.sync.dma_start(out=xt[:], in_=xT_p[128 * kd:128 * (kd + 1), cs])
                    xts.append(xt)
                qc = [qpool.tile([128, 512], F32R, tag='qc', name=f'qc{c}_{m}')
                      for m in range(4)]
                vT = vpool.tile([128, 512], F32, tag='vT')
                for m in range(6):
                    ps = accp.tile([128, 512], F32, tag='acc', name=f'ps{c}_{m}')
                    for kd in range(KD):
                        nc.tensor.matmul(ps[:], WT[:, kd * 768 + 128 * m: kd * 768 + 128 * (m + 1)],
                                         xts[kd][:], start=(kd == 0), stop=(kd == KD - 1))
                    if m < 5:
                        # RoPE: out = ps*cos4 + swap32(ps)*sin4 (sign baked in sin4)
                        t1 = tpool.tile([128, 512], F32, tag='t1')
                        t2 = tpool.tile([128, 512], F32, tag='t2')
                        nc.vector.tensor_mul(t1[:], ps[:], cos4[:, cs])
                        for g in range(2):
                            b0 = 64 * g
                            nc.vector.tensor_mul(t2[b0:b0 + 32, :], ps[b0 + 32:b0 + 64, :],
                                                 sin4[b0:b0 + 32, cs])
                            nc.vector.tensor_mul(t2[b0 + 32:b0 + 64, :], ps[b0:b0 + 32, :],
                                                 sin4[b0 + 32:b0 + 64, cs])
                        tgt = qc[m][:] if m < 4 else k4[:, cs]
                        nc.vector.tensor_add(tgt, t1[:], t2[:])
                    else:
                        nc.scalar.copy(vT[:], ps[:])
                # kT replication for chunk c: rep0=[kv0|kv0], rep1=[kv1|kv1]
                nc.sync.dma_start(out=kswap[0:64, cs], in_=k4[64:128, cs])
                nc.sync.dma_start(out=kswap[64:128, cs], in_=k4[0:64, cs])
                # V natural (bf16, ones-augmented): per j [v0 64|1|v1 64|1]
                for j in range(4 * c, 4 * c + 4):
                    pt = accp.tile([128, 128], F32, tag='acc', name=f'pt{j}')
                    nc.tensor.transpose(pt[:], vT[:, 128 * (j - 4 * c):128 * (j - 4 * c) + 128], ident[:])
                    nc.vector.tensor_copy(V[:, 130 * j:130 * j + 64], pt[:, 0:64])
                    nc.vector.tensor_copy(V[:, 130 * j + 65:130 * j + 129], pt[:, 64:128])
                if c == 0:
                    for hc in range(4):
                        nc.sync.dma_start(out=WoT[:, hc * D:(hc + 1) * D],
                                          in_=WoT_p[128 * hc:128 * (hc + 1), :])

                # ---- attention for chunk c (+ interleaved outproj of c-1) ----
                nj = 4 * c + 4
                for sb in range(4 * (c - 1), 4 * c) if c >= 1 else ():
                    for dg in range(2):
                        pending.append((c - 1, sb, dg))
                for hp in range(4):
                    kv = hp // 2
                    vbase = 65 * kv
                    spread = {max(1, nj // 3), max(2, (2 * nj) // 3)}
                    oAB = pop.tile([65, 1024], F32, tag='oab')
                    atas = {}

                    def scores(j):
                        sct = scp.tile([128, 1024], F32, tag='sc', name=f'sc{hp}_{c}_{j}')
                        if kv == 0:
                            kA = k4[0:64, 128 * j:128 * (j + 1)]
                            kB = kswap[64:128, 128 * j:128 * (j + 1)]
                        else:
                            kA = kswap[0:64, 128 * j:128 * (j + 1)]
                            kB = k4[64:128, 128 * j:128 * (j + 1)]
                        nc.tensor.matmul(sct[:, 0:512], kA, qc[hp][0:64, :], start=True, stop=True)
                        nc.tensor.matmul(sct[:, 512:1024], kB, qc[hp][64:128, :], start=True, stop=True)
                        ata = atpool.tile([128, 1024], BF16, tag='at', name=f'at{hp}_{c}_{j}')
                        nc.scalar.activation(ata[:], sct[:], mybir.ActivationFunctionType.Exp,
                                             scale=SCALE)
                        d = j - 4 * c
                        if 0 <= d <= 3:
                            nc.vector.tensor_mul(ata[:, 0:512], ata[:, 0:512],
                                                 masks[:, 512 * d:512 * (d + 1)])
                            nc.gpsimd.tensor_mul(ata[:, 512:1024], ata[:, 512:1024],
                                                 masks[:, 512 * d:512 * (d + 1)])
                        atas[j] = ata

                    def attnv(j):
                        ata = atas.pop(j)
                        nc.tensor.matmul(oAB[:, 0:512], V[:, 130 * j + vbase:130 * j + vbase + 65],
                                         ata[:, 0:512], start=(j == 0), stop=(j == nj - 1))
                        nc.tensor.matmul(oAB[:, 512:1024], V[:, 130 * j + vbase:130 * j + vbase + 65],
                                         ata[:, 512:1024], start=(j == 0), stop=(j == nj - 1))

                    scores(0)
                    for j in range(1, nj):
                        scores(j)
                        attnv(j - 1)
                        if j in spread and pending:
                            outproj_unit(*pending.pop(0))
                    attnv(nj - 1)

                    # normalize: evacuate PSUM fast, then recip/broadcast/mul
                    sbAB = spool.tile([65, 1024], F32, tag='sbab')
                    nc.vector.tensor_copy(sbAB[:], oAB[:])
                    rAB = spool.tile([1, 1024], F32, tag='rab')
                    nc.vector.reciprocal(rAB[:], sbAB[64:65, :])
                    bAB = spool.tile([64, 1024], F32, tag='bab')
                    nc.gpsimd.partition_broadcast(bAB[:], rAB[0:1, :])
                    ao = aopool.tile([128, 512], F16, tag='ao', name=f'ao{c}_{hp}')
                    aout_c[(c, hp)] = ao
                    nc.vector.tensor_mul(ao[0:64, :], sbAB[0:64, 0:512], bAB[:, 0:512])
                    nc.vector.tensor_mul(ao[64:128, :], sbAB[0:64, 512:1024], bAB[:, 512:1024])

            for sb in range(12, 16):
                for dg in range(2):
                    pending.append((3, sb, dg))
            while pending:
                outproj_unit(*pending.pop(0))
    nc.compile()
    return nc


_PERM = np.concatenate([np.arange(0, DH, 2), np.arange(1, DH, 2)])


def _prep_core(x, Wq, Wk, Wv, Wo, cos, sin, b, hg):
    xT = np.ascontiguousarray(x[b].T.astype(np.float32))
    # q heads 8hg..8hg+7 permuted, kv heads 2hg,2hg+1 (k permuted, v natural)
    wq = Wq.reshape(H, DH, D)[8 * hg:8 * hg + 8][:, _PERM, :].reshape(512, D)
    wk = Wk.reshape(KVH, DH, D)[2 * hg:2 * hg + 2][:, _PERM, :].reshape(128, D)
    wv = Wv.reshape(KVH, DH, D)[2 * hg:2 * hg + 2].reshape(128, D)
    WT = np.ascontiguousarray(np.concatenate([wq, wk, wv], 0).T.astype(np.float32))
    WoT = np.ascontiguousarray(Wo[:, 512 * hg:512 * (hg + 1)].T.astype(np.float16))
    cosT = np.ascontiguousarray(cos.T.astype(np.float32))          # (32, S)
    sinT = np.ascontiguousarray(sin.T.astype(np.float32))
    cos4 = np.tile(cosT, (4, 1)).astype(np.float16)
    sin4 = np.concatenate([-sinT, sinT, -sinT, sinT], 0).astype(np.float16)
    mask = np.zeros((128, 4 * 512), dtype=np.float64)
    for dd in range(4):
        mask[:, 512 * dd:512 * (dd + 1)] = \
            (128 * dd + np.arange(128)[:, None]) <= np.arange(512)[None, :]
    return {'xT': xT, 'WT': WT, 'WoT': WoT, 'cos4': cos4, 'sin4': sin4,
            'mask': mask.astype(ml_dtypes.bfloat16),
            'ident': np.eye(128, dtype=np.float32)}


def _run(inputs, trace=False, tmpdir=None):
    if 'nc' not in _cache:
        _cache['nc'] = build()
    in_maps = [_prep_core(inputs['x'], inputs['Wq'], inputs['Wk'], inputs['Wv'],
                          inputs['Wo'], inputs['cos'], inputs['sin'], c // 4, c % 4)
               for c in range(8)]
    res = run_bass_kernel_spmd(_cache['nc'], in_maps, core_ids=list(range(8)),
                               trace=trace, tmpdir=tmpdir)
    parts = [res.results[c]['out'] for c in range(8)]
    out = np.stack([parts[0] + parts[1] + parts[2] + parts[3],
                    parts[4] + parts[5] + parts[6] + parts[7]], 0)
    return out.astype(np.float32), res


def kernel(**inputs):
    out, _ = _run(inputs, trace=False)
    return out


# revision 13
# speedup vs baseline: 2.0634x; 1.4403x over previous
"""GQA kernel for trn2: 8 NeuronCores, SPMD (b in {0,1} x 4 head-groups).

Per core (b, hg): 8 q-heads (8hg..8hg+7), 2 kv-heads (2hg, 2hg+1).
c-major software pipeline: per 512-wide q chunk c emit
  proj(c) -> RoPE -> V-build -> attention(c) with outproj(c-1) interleaved
so PE never sees a phase barrier (keeps HAM un-throttled).
f32r matmuls for proj/scores, bf16 for attn@V, fp16 for outproj.
Host preps transposed/permuted weights; partial outputs summed on host
(row-parallel Wo all-reduce done during unshard).
"""
import numpy as np
import ml_dtypes
import concourse.bass as bass
import concourse.mybir as mybir
from concourse import tile, bacc
from concourse.bass_utils import run_bass_kernel_spmd

B, S, D = 2, 2048, 2048
H, KVH, DH = 32, 8, 64
SCALE = DH ** -0.5
SC = 4          # Sq chunks of 512
KD = 16         # D contraction chunks of 128
NJ = 16         # Sk blocks of 128
F32R = mybir.dt.float32r
F32 = mybir.dt.float32
BF16 = mybir.dt.bfloat16
F16 = mybir.dt.float16

_cache = {}


def build():
    nc = bacc.Bacc('TRN2', target_bir_lowering=False, debug=False, num_devices=8)
    xT_p = nc.declare_dram_parameter('xT', [D, S], F32R, isOutput=False)
    WT_p = nc.declare_dram_parameter('WT', [D, 768], F32R, isOutput=False)
    WoT_p = nc.declare_dram_parameter('WoT', [512, D], F16, isOutput=False)
    cos4_p = nc.declare_dram_parameter('cos4', [128, S], F16, isOutput=False)
    sin4_p = nc.declare_dram_parameter('sin4', [128, S], F16, isOutput=False)
    mask_p = nc.declare_dram_parameter('mask', [128, 4 * 512], BF16, isOutput=False)
    ident_p = nc.declare_dram_parameter('ident', [128, 128], F32, isOutput=False)
    out_p = nc.declare_dram_parameter('out', [S, D], F32, isOutput=True)

    with tile.TileContext(nc) as tc:
        with tc.tile_pool(name='w', bufs=1) as wpool, \
             tc.tile_pool(name='x', bufs=18) as xpool, \
             tc.tile_pool(name='q', bufs=8) as qpool, \
             tc.tile_pool(name='ao', bufs=8) as aopool, \
             tc.tile_pool(name='v', bufs=2) as vpool, \
             tc.tile_pool(name='t', bufs=2) as tpool, \
             tc.tile_pool(name='at', bufs=3) as atpool, \
             tc.tile_pool(name='s', bufs=1) as spool, \
             tc.tile_pool(name='o', bufs=2) as opool, \
             tc.tile_pool(name='acc', bufs=2, space='PSUM') as accp, \
             tc.tile_pool(name='sc', bufs=2, space='PSUM') as scp, \
             tc.tile_pool(name='po', bufs=1, space='PSUM') as pop:

            WT = wpool.tile([128, KD * 768], F32R, tag='WT')
            for kd in range(KD):
                nc.sync.dma_start(out=WT[:, kd * 768:(kd + 1) * 768],
                                  in_=WT_p[128 * kd:128 * (kd + 1), :])
            cos4 = wpool.tile([128, S], F16, tag='cos4')
            sin4 = wpool.tile([128, S], F16, tag='sin4')
            masks = wpool.tile([128, 4 * 512], BF16, tag='masks')
            ident = wpool.tile([128, 128], F32, tag='ident')
            nc.sync.dma_start(out=cos4[:], in_=cos4_p[:])
            nc.sync.dma_start(out=sin4[:], in_=sin4_p[:])
            nc.sync.dma_start(out=masks[:], in_=mask_p[:])
            nc.sync.dma_start(out=ident[:], in_=ident_p[:])

            k4 = wpool.tile([128, S], F32R, tag='k4')
            kswap = wpool.tile([128, S], F32R, tag='kswap')
            V = wpool.tile([128, NJ * 130], BF16, tag='V')
            nc.vector.memset(V[:], 1.0)
            WoT = wpool.tile([128, 4 * D], F16, tag='WoT')

            aout_c = {}   # (c, hp) -> per-chunk fp16 attention-output tile

            def outproj_unit(cc, sb, dg):
                # one (s-block, 1024-wide D group): 8 matmuls + 2 evacs + 1 DMA
                ost = opool.tile([128, 1024], F32, tag='ost')
                for i, dc in enumerate((2 * dg, 2 * dg + 1)):
                    po = accp.tile([128, 512], F32, tag='acc', name=f'po{sb}_{dc}')
                    for hc in range(4):
                        nc.tensor.matmul(po[:], aout_c[(cc, hc)][:, 128 * (sb - 4 * cc):128 * (sb - 4 * cc) + 128],
                                         WoT[:, hc * D + 512 * dc: hc * D + 512 * (dc + 1)],
                                         start=(hc == 0), stop=(hc == 3))
                    if dc % 2 == 0:
                        nc.scalar.copy(ost[:, 512 * i:512 * (i + 1)], po[:])
                    else:
                        nc.vector.tensor_copy(ost[:, 512 * i:512 * (i + 1)], po[:])
                nc.sync.dma_start(out=out_p[128 * sb:128 * (sb + 1), 1024 * dg:1024 * (dg + 1)],
                                  in_=ost[:])

            pending = []  # deferred outproj units of the previous chunk

            for c in range(SC):
                cs = slice(512 * c, 512 * (c + 1))
                # ---- projections + RoPE for chunk c ----
                xts = []
                for kd in range(KD):
                    xt = xpool.tile([128, 512], F32R, tag='xt')
                    nc.sync.dma_start(out=xt[:], in_=xT_p[128 * kd:128 * (kd + 1), cs])
                    xts.append(xt)
                qc = [qpool.tile([128, 512], F32R, tag='qc', name=f'qc{c}_{m}')
                      for m in range(4)]
                vT = vpool.tile([128, 512], F32, tag='vT')
                for m in range(6):
                    ps = accp.tile([128, 512], F32, tag='acc', name=f'ps{c}_{m}')
                    for kd in range(KD):
                        nc.tensor.matmul(ps[:], WT[:, kd * 768 + 128 * m: kd * 768 + 128 * (m + 1)],
                                         xts[kd][:], start=(kd == 0), stop=(kd == KD - 1))
                    if m < 5:
                        # RoPE: out = ps*cos4 + swap32(ps)*sin4 (sign baked in sin4)
                        t1 = tpool.tile([128, 512], F32, tag='t1')
                        t2 = tpool.tile([128, 512], F32, tag='t2')
                        nc.vector.tensor_mul(t1[:], ps[:], cos4[:, cs])
                        for g in range(2):
                            b0 = 64 * g
                            nc.vector.tensor_mul(t2[b0:b0 + 32, :], ps[b0 + 32:b0 + 64, :],
                                                 sin4[b0:b0 + 32, cs])
                            nc.vector.tensor_mul(t2[b0 + 32:b0 + 64, :], ps[b0:b0 + 32, :],
                                                 sin4[b0 + 32:b0 + 64, cs])
                        tgt = qc[m][:] if m < 4 else k4[:, cs]
                        nc.vector.tensor_add(tgt, t1[:], t2[:])
                    else:
                        nc.scalar.copy(vT[:], ps[:])
                # kT replication for chunk c: rep0=[kv0|kv0], rep1=[kv1|kv1]
                nc.sync.dma_start(out=kswap[0:64, cs], in_=k4[64:128, cs])
                nc.sync.dma_start(out=kswap[64:128, cs], in_=k4[0:64, cs])
                # V natural (bf16, ones-augmented): per j [v0 64|1|v1 64|1]
                for j in range(4 * c, 4 * c + 4):
                    pt = accp.tile([128, 128], F32, tag='acc', name=f'pt{j}')
                    nc.tensor.transpose(pt[:], vT[:, 128 * (j - 4 * c):128 * (j - 4 * c) + 128], ident[:])
                    nc.vector.tensor_copy(V[:, 130 * j:130 * j + 64], pt[:, 0:64])
                    nc.vector.tensor_copy(V[:, 130 * j + 65:130 * j + 129], pt[:, 64:128])
                if c == 0:
                    for hc in range(4):
                        nc.sync.dma_start(out=WoT[:, hc * D:(hc + 1) * D],
                                          in_=WoT_p[128 * hc:128 * (hc + 1), :])

                # ---- attention for chunk c (+ interleaved outproj of c-1) ----
                nj = 4 * c + 4
                for sb in range(4 * (c - 1), 4 * c) if c >= 1 else ():
                    for dg in range(2):
                        pending.append((c - 1, sb, dg))
                for hp in range(4):
                    kv = hp // 2
                    vbase = 65 * kv
                    spread = {max(1, nj // 3), max(2, (2 * nj) // 3)}
                    oAB = pop.tile([65, 1024], F32, tag='oab')
                    atas = {}

                    def scores(j):
                        sct = scp.tile([128, 1024], F32, tag='sc', name=f'sc{hp}_{c}_{j}')
                        if kv == 0:
                            kA = k4[0:64, 128 * j:128 * (j + 1)]
                            kB = kswap[64:128, 128 * j:128 * (j + 1)]
                        else:
                            kA = kswap[0:64, 128 * j:128 * (j + 1)]
                            kB = k4[64:128, 128 * j:128 * (j + 1)]
                        nc.tensor.matmul(sct[:, 0:512], kA, qc[hp][0:64, :], start=True, stop=True)
                        nc.tensor.matmul(sct[:, 512:1024], kB, qc[hp][64:128, :], start=True, stop=True)
                        ata = atpool.tile([128, 1024], BF16, tag='at', name=f'at{hp}_{c}_{j}')
                        nc.scalar.activation(ata[:], sct[:], mybir.ActivationFunctionType.Exp,
                                             scale=SCALE)
                        d = j - 4 * c
                        if 0 <= d <= 3:
                            nc.vector.tensor_mul(ata[:, 0:512], ata[:, 0:512],
                                                 masks[:, 512 * d:512 * (d + 1)])
                            nc.vector.tensor_mul(ata[:, 512:1024], ata[:, 512:1024],
                                                 masks[:, 512 * d:512 * (d + 1)])
                        atas[j] = ata

                    def attnv(j):
                        ata = atas.pop(j)
                        nc.tensor.matmul(oAB[:, 0:512], V[:, 130 * j + vbase:130 * j + vbase + 65],
                                         ata[:, 0:512], start=(j == 0), stop=(j == nj - 1))
                        nc.tensor.matmul(oAB[:, 512:1024], V[:, 130 * j + vbase:130 * j + vbase + 65],
                                         ata[:, 512:1024], start=(j == 0), stop=(j == nj - 1))

                    scores(0)
                    for j in range(1, nj):
                        scores(j)
                        attnv(j - 1)
                        if j in spread and pending:
                            outproj_unit(*pending.pop(0))
                    attnv(nj - 1)

                    # normalize: evacuate PSUM fast, then recip/broadcast/mul
                    # (denominator row copied to partition 0 for the custom DVE recip)
                    d0 = spool.tile([1, 1024], F32, tag='d0')
                    nc.scalar.copy(d0[:], oAB[64:65, :])
                    sbAB = spool.tile([64, 1024], F32, tag='sbab')
                    nc.scalar.copy(sbAB[:], oAB[0:64, :])
                    rAB = spool.tile([1, 1024], F32, tag='rab')
                    nc.vector.reciprocal_approx_fast(rAB[:], d0[:])
                    bAB = spool.tile([64, 1024], F32, tag='bab')
                    nc.gpsimd.partition_broadcast(bAB[:], rAB[0:1, :])
                    ao = aopool.tile([128, 512], F16, tag='ao', name=f'ao{c}_{hp}')
                    aout_c[(c, hp)] = ao
                    nc.vector.tensor_mul(ao[0:64, :], sbAB[:, 0:512], bAB[:, 0:512])
                    nc.vector.tensor_mul(ao[64:128, :], sbAB[:, 512:1024], bAB[:, 512:1024])

            for sb in range(12, 16):
                for dg in range(2):
                    pending.append((3, sb, dg))
            while pending:
                outproj_unit(*pending.pop(0))
    nc.compile()
    return nc


_PERM = np.concatenate([np.arange(0, DH, 2), np.arange(1, DH, 2)])


def _prep_core(x, Wq, Wk, Wv, Wo, cos, sin, b, hg):
    xT = np.ascontiguousarray(x[b].T.astype(np.float32))
    # q heads 8hg..8hg+7 permuted, kv heads 2hg,2hg+1 (k permuted, v natural)
    wq = Wq.reshape(H, DH, D)[8 * hg:8 * hg + 8][:, _PERM, :].reshape(512, D)
    wk = Wk.reshape(KVH, DH, D)[2 * hg:2 * hg + 2][:, _PERM, :].reshape(128, D)
    wv = Wv.reshape(KVH, DH, D)[2 * hg:2 * hg + 2].reshape(128, D)
    WT = np.ascontiguousarray(np.concatenate([wq, wk, wv], 0).T.astype(np.float32))
    WoT = np.ascontiguousarray(Wo[:, 512 * hg:512 * (hg + 1)].T.astype(np.float16))
    cosT = np.ascontiguousarray(cos.T.astype(np.float32))          # (32, S)
    sinT = np.ascontiguousarray(sin.T.astype(np.float32))
    cos4 = np.tile(cosT, (4, 1)).astype(np.float16)
    sin4 = np.concatenate([-sinT, sinT, -sinT, sinT], 0).astype(np.float16)
    mask = np.zeros((128, 4 * 512), dtype=np.float64)
    for dd in range(4):
        mask[:, 512 * dd:512 * (dd + 1)] = \
            (128 * dd + np.arange(128)[:, None]) <= np.arange(512)[None, :]
    return {'xT': xT, 'WT': WT, 'WoT': WoT, 'cos4': cos4, 'sin4': sin4,
            'mask': mask.astype(ml_dtypes.bfloat16),
            'ident': np.eye(128, dtype=np.float32)}


def _run(inputs, trace=False, tmpdir=None):
    if 'nc' not in _cache:
        _cache['nc'] = build()
    in_maps = [_prep_core(inputs['x'], inputs['Wq'], inputs['Wk'], inputs['Wv'],
                          inputs['Wo'], inputs['cos'], inputs['sin'], c // 4, c % 4)
               for c in range(8)]
    res = run_bass_kernel_spmd(_cache['nc'], in_maps, core_ids=list(range(8)),
                               trace=trace, tmpdir=tmpdir)
    parts = [res.results[c]['out'] for c in range(8)]
    out = np.stack([parts[0] + parts[1] + parts[2] + parts[3],
                    parts[4] + parts[5] + parts[6] + parts[7]], 0)
    return out.astype(np.float32), res


def kernel(**inputs):
    out, _ = _run(inputs, trace=False)
    return out
